# revision 1
# baseline (speedup 1.0000x reference)
"""Trainium2 Bass kernel for nn_Block_69191923139027 (dense_transformer).

Sharding: 8 cores; core k handles Feebler/Booster rows i in [8k, 8k+8) for
all batches. AllGather collectives stitch the per-batch global reductions
(ck/cv, softmax denominator) and the full h_final needed by the Booster.

v3: merged 4MB DMAs (x/fw/bw/out via strided APs), per-batch pipelines on
both sides (feebler->LN1->qkv and proj->LN2->FFN->tok), softmax scale
folded into ACT-exp scale and proj weights, rstd = exp(-0.5*ln(var+eps)),
bf16 feebler-reduce matmuls, constant-shift softmax (max |logit| ~71 < 88).

Self-contained: hardcodes all shapes; no sibling imports.
"""

import numpy as np

import concourse.bacc as bacc
import concourse.mybir as mybir
import concourse.tile as tile
from concourse.bass_utils import run_bass_kernel_spmd

N_CORES = 8
B, T, SD, NE = 4, 2048, 64, 4096
H, HS, FH = 8, 8, 256
EPS = 1e-5
IPC = SD // N_CORES          # 8 feebler rows per core
TLOC = B * IPC * 32          # 1024 local tokens; hT col = b*256 + a*8 + i
DT = mybir.dt.float32
BF = mybir.dt.bfloat16
RG = [list(range(N_CORES))]
ESHIFT = 64.0                # softmax logit shift (max |logit| ~ 71)

_CACHE = {}


def _build_nc():
    nc = bacc.Bacc("TRN2", target_bir_lowering=False, debug=False,
                   num_devices=N_CORES)
    A = mybir.AluOpType
    AF = mybir.ActivationFunctionType

    tn = {}
    tn["x"] = nc.dram_tensor("x", [B * IPC * SD, T], DT, kind="ExternalInput")
    tn["fw"] = nc.dram_tensor("fw", [IPC * SD, T], DT, kind="ExternalInput")
    tn["bw"] = nc.dram_tensor("bw", [IPC * SD, T], DT, kind="ExternalInput")
    tn["wqkv"] = nc.dram_tensor("wqkv", [SD, 3 * SD], DT, kind="ExternalInput")
    tn["pw"] = nc.dram_tensor("pw", [SD, SD], DT, kind="ExternalInput")
    tn["pb"] = nc.dram_tensor("pb", [SD, 1], DT, kind="ExternalInput")
    tn["l1g"] = nc.dram_tensor("l1g", [SD, 1], DT, kind="ExternalInput")
    tn["l1b"] = nc.dram_tensor("l1b", [SD, 1], DT, kind="ExternalInput")
    tn["l2g"] = nc.dram_tensor("l2g", [SD, 1], DT, kind="ExternalInput")
    tn["l2b"] = nc.dram_tensor("l2b", [SD, 1], DT, kind="ExternalInput")
    tn["w1"] = nc.dram_tensor("w1", [SD, FH], DT, kind="ExternalInput")
    tn["b1h"] = nc.dram_tensor("b1h", [128, 2], DT, kind="ExternalInput")
    tn["w2"] = nc.dram_tensor("w2", [FH, SD], DT, kind="ExternalInput")
    tn["b2"] = nc.dram_tensor("b2", [SD, 1], DT, kind="ExternalInput")
    tn["eye64"] = nc.dram_tensor("eye64", [64, 64], DT, kind="ExternalInput")
    out = nc.dram_tensor("out", [B * IPC * SD, T], DT, kind="ExternalOutput")

    with tile.TileContext(nc) as tc:
        _body(nc, tc, tn, out, A, AF)
    nc.compile()
    return nc


def _body(nc, tc, tn, out, A, AF):
    X = mybir.AxisListType.X
    T4 = 4 * T  # 8192

    with tc.tile_pool(name="wconst", bufs=1) as wp, \
         tc.tile_pool(name="mid", bufs=1) as mp, \
         tc.tile_pool(name="bwpool", bufs=1) as bwp, \
         tc.tile_pool(name="dram", bufs=1, space="DRAM") as dp:

        # ---- on-chip constants (no DMA traffic) ----
        ones2 = wp.tile([128, 2], DT, tag="ones2")
        nc.vector.memset(ones2[:], 0.0)
        nc.vector.memset(ones2[0:64, 0:1], 1.0)
        nc.vector.memset(ones2[64:128, 1:2], 1.0)
        ones64 = wp.tile([SD, 1], DT, tag="ones64")
        nc.vector.memset(ones64[:], 1.0 / SD)
        epsv = wp.tile([64, 1], DT, tag="epsv")
        nc.vector.memset(epsv[:], EPS)
        neg64 = wp.tile([64, 1], DT, tag="neg64")
        nc.vector.memset(neg64[:], -ESHIFT)

        hTb = [mp.tile([64, 256], DT, tag=f"hT{b}", name=f"hT{b}")
               for b in range(B)]
        y1b = [mp.tile([64, 256], DT, tag=f"y1{b}", name=f"y1{b}")
               for b in range(B)]
        eTb = [mp.tile([64, 256], DT, tag=f"eT{b}", name=f"eT{b}")
               for b in range(B)]
        part = mp.tile([64, 4], DT, tag="part")   # ysum partials per batch

        wtiles = {}

        def wtile(name, shape, src):
            t = wp.tile(shape, DT, tag=name, name=f"w_{name}")
            nc.sync.dma_start(t[:], src)
            wtiles[name] = t
            return t

        def emit_small_weights():
            wtile("wqkv", [SD, 3 * SD], tn["wqkv"][:])
            wtile("pw", [SD, SD], tn["pw"][:])
            wtile("pb", [SD, 1], tn["pb"][:])
            wtile("l1g", [SD, 1], tn["l1g"][:])
            wtile("l1b", [SD, 1], tn["l1b"][:])
            wtile("l2g", [SD, 1], tn["l2g"][:])
            wtile("l2b", [SD, 1], tn["l2b"][:])
            wtile("w1", [SD, FH], tn["w1"][:])
            wtile("b1h", [128, 2], tn["b1h"][:])
            wtile("w2a", [128, SD], tn["w2"][0:128, :])
            wtile("w2b", [128, SD], tn["w2"][128:256, :])
            wtile("b2", [SD, 1], tn["b2"][:])
            wtile("eye64", [64, 64], tn["eye64"][:])

        # layer norm on [64, W] slice; aux tiles from given pools.
        # rstd = exp(-0.5*ln(var+eps)) keeps the hot path on ACT.
        def layer_norm(y_out, h_ap, g, bta, W, pls, lnp):
            sq = lnp.tile([64, W], DT, tag="ln_sq")
            nc.vector.tensor_mul(sq[:], h_ap, h_ap)
            stats_ps = pls.tile([1, 2 * W], DT, tag="ln_stats", bufs=2)
            mean_ps = stats_ps[:, 0:W]
            msq_ps = stats_ps[:, W:2 * W]
            for c in range(0, W, 512):
                sl = slice(c, min(c + 512, W))
                slm = slice(W + c, W + min(c + 512, W))
                nc.tensor.matmul(stats_ps[:, sl], ones64[:], h_ap[:, sl],
                                 start=True, stop=True)
                nc.tensor.matmul(stats_ps[:, slm], ones64[:], sq[:, sl],
                                 start=True, stop=True)
            mean_sb = lnp.tile([1, W], DT, tag="ln_mean_sb")
            nc.scalar.copy(mean_sb[:], mean_ps)
            mbsq = lnp.tile([1, W], DT, tag="ln_mbsq")
            nc.scalar.square(mbsq[:], mean_ps)
            var = lnp.tile([1, W], DT, tag="ln_var")
            nc.vector.tensor_sub(var[:], msq_ps, mbsq[:])
            lnv = lnp.tile([1, W], DT, tag="ln_lnv")
            nc.scalar.activation(lnv[:], var[:], AF.Ln, bias=epsv[0:1, 0:1])
            rstd = lnp.tile([1, W], DT, tag="ln_rstd")
            nc.scalar.activation(rstd[:], lnv[:], AF.Exp, scale=-0.5)
            meanb = lnp.tile([64, W], DT, tag="ln_meanb")
            nc.gpsimd.partition_broadcast(meanb[:], mean_sb[:])
            rstdb = lnp.tile([64, W], DT, tag="ln_rstdb")
            nc.gpsimd.partition_broadcast(rstdb[:], rstd[:])
            ymm = lnp.tile([64, W], DT, tag="ln_ymm")
            nc.vector.tensor_sub(ymm[:], h_ap, meanb[:])
            nc.vector.scalar_tensor_tensor(y_out, ymm[:], g[:, 0:1], rstdb[:],
                                           op0=A.mult, op1=A.mult)
            nc.vector.tensor_scalar_add(y_out, y_out, bta[:, 0:1])

        # ======== Phase A+B1: per-batch feebler -> transpose -> LN1 -> qkv
        with nc.named_scope("feebler"), \
             tc.tile_pool(name="fw", bufs=1) as fwp, \
             tc.tile_pool(name="xin", bufs=2) as xp, \
             tc.tile_pool(name="prod", bufs=2) as prp, \
             tc.tile_pool(name="ln1t", bufs=2) as lnp1, \
             tc.tile_pool(name="psAB", bufs=1, space="PSUM") as psAB:
            fwt = fwp.tile([128, T4], DT, tag="fwt")
            for b in range(B):
                xt = xp.tile([128, T4], DT, tag="x")
                if b == 0:
                    # interleave fw/x 1MB chunks so compute starts early
                    for m in range(4):
                        msl = slice(m * T, (m + 1) * T)
                        nc.sync.dma_start(
                            fwt[:, msl],
                            tn["fw"][2 * m * 64:(2 * m + 2) * 64, :])
                        nc.sync.dma_start(
                            xt[:, msl], tn["x"][m * 128:(m + 1) * 128, :])
                else:
                    nc.sync.dma_start(
                        xt[:], tn["x"][b * 512:(b + 1) * 512, :].rearrange(
                            "(m p) t -> p m t", m=4))
                dstv_all = hTb[b][:].rearrange("p (c e) -> p c e", c=16)
                for m in range(4):
                    msl = slice(m * T, (m + 1) * T)
                    prbf = prp.tile([128, T], DT, tag="prbf")
                    nc.vector.tensor_mul(prbf[:], xt[:, msl], fwt[:, msl])
                    # j-reduce with output already transposed: out rows are
                    # (a-parity, s), cols are the i-pair
                    hT2 = psAB.tile([128, 32], DT, tag="hT2", bufs=2)
                    for c in range(16):
                        nc.tensor.matmul(hT2[:, 2 * c:2 * c + 2],
                                         prbf[:, c * 128:(c + 1) * 128],
                                         ones2[:], start=True, stop=True)
                    for apar in range(2):
                        srcv = hT2[apar * 64:apar * 64 + 64, :].rearrange(
                            "p (c i) -> p c i", c=16)
                        e0 = 8 * apar + 2 * m
                        nc.scalar.copy(dstv_all[:, :, e0:e0 + 2], srcv)
                if b == 0:
                    emit_small_weights()
                if b == 3:
                    pass
                # -- LN1(b); k/v global sums come from AllGather(sum y)
                # via linearity: ck = wk^T @ sum(y), cv = wv^T @ sum(y)
                layer_norm(y1b[b][:], hTb[b][:], wtiles["l1g"],
                           wtiles["l1b"], 256, psAB, lnp1)
                nc.vector.tensor_reduce(part[:, b:b + 1], y1b[b][:],
                                        axis=X, op=A.add)

        # ======== AG1: y-sum partials -> global; q matmul overlaps it
        cc1i = dp.tile([64, 4], DT, tag="cc1i")
        cc1o = dp.tile([512, 4], DT, tag="cc1o", addr_space="Shared")
        nc.sync.dma_start(cc1i[:], part[:])
        nc.gpsimd.collective_compute("AllGather", A.bypass, ins=[cc1i[:]],
                                     outs=[cc1o[:]], replica_groups=RG)
        zp = mp.tile([64, 4], DT, tag="zp")
        with nc.named_scope("softmax"), \
             tc.tile_pool(name="psQ", bufs=1, space="PSUM") as psq:
            q_ps = psq.tile([64, TLOC], DT, tag="q_ps")
            for b in range(B):
                sl = slice(b * 256, (b + 1) * 256)
                nc.tensor.matmul(q_ps[:, sl], wtiles["wqkv"][:, 0:64],
                                 y1b[b][:], start=True, stop=True)
            gath = mp.tile([64, 32], DT, tag="gath")   # (batch 4, rank 8)
            nc.sync.dma_start(
                gath[:].rearrange("p (s r) -> p s r", s=4),
                cc1o[:].rearrange("(r p) s -> p s r", r=N_CORES))
            ysum = mp.tile([64, 4], DT, tag="ysum")
            nc.vector.tensor_reduce(ysum[:],
                                    gath[:].rearrange("p (s r) -> p s r",
                                                      s=4),
                                    axis=X, op=A.add)
            kv_ps = psq.tile([128, 4], DT, tag="kv_ps")
            nc.tensor.matmul(kv_ps[:], wtiles["wqkv"][:, 64:192], ysum[:],
                             start=True, stop=True)
            kvg = mp.tile([128, 4], DT, tag="kvg")   # ck rows 0:64, cv 64:128
            nc.scalar.copy(kvg[:], kv_ps[:])
            # e = exp(q*ck - 64); accumulate local softmax denominator
            for b in range(B):
                sl = slice(b * 256, (b + 1) * 256)
                nc.scalar.activation(eTb[b][:], q_ps[:, sl], AF.Exp,
                                     bias=neg64[:, 0:1],
                                     scale=kvg[0:64, b:b + 1],
                                     accum_out=zp[:, b:b + 1])
        # AG2: softmax denominator
        cc2i = dp.tile([64, 4], DT, tag="cc2i")
        cc2o = dp.tile([512, 4], DT, tag="cc2o", addr_space="Shared")
        nc.sync.dma_start(cc2i[:], zp[:])
        nc.gpsimd.collective_compute("AllGather", A.bypass, ins=[cc2i[:]],
                                     outs=[cc2o[:]], replica_groups=RG)
        gath2 = mp.tile([64, 32], DT, tag="gath2")
        nc.sync.dma_start(gath2[:].rearrange("p (s r) -> p s r", s=4),
                          cc2o[:].rearrange("(r p) s -> p s r", r=N_CORES))
        zg = mp.tile([64, 4], DT, tag="zg")
        nc.vector.tensor_reduce(zg[:],
                                gath2[:].rearrange("p (s r) -> p s r", s=4),
                                axis=X, op=A.add)
        rz = mp.tile([64, 4], DT, tag="rz")
        nc.vector.reciprocal(rz[:], zg[:])
        cvg = mp.tile([64, 4], DT, tag="cvg")
        nc.scalar.copy(cvg[:], kvg[64:128, :])
        sc = mp.tile([64, 4], DT, tag="sc")
        nc.vector.tensor_mul(sc[:], cvg[:], rz[:])

        # ======== per-batch: proj (sc folded into weights) -> LN2 -> FFN
        # ======== -> token-major transposes -> split AllGather
        cc3i = [dp.tile([8, 2048], DT, tag=f"cc3i{g}", name=f"cc3i{g}")
                for g in range(B)]
        cc3o = [dp.tile([8, 16384], DT, tag=f"cc3o{g}", name=f"cc3o{g}",
                        addr_space="Shared") for g in range(B)]
        with nc.named_scope("post"), \
             tc.tile_pool(name="postt", bufs=2) as pot, \
             tc.tile_pool(name="psPO", bufs=1, space="PSUM") as pps:
            eye64 = wtiles["eye64"]
            for b in range(B):
                pwb = pot.tile([64, 64], DT, tag="pwb")
                nc.vector.tensor_scalar_mul(pwb[:], wtiles["pw"][:],
                                            sc[:, b:b + 1])
                pj = pps.tile([64, 256], DT, tag="pj", bufs=2)
                nc.tensor.matmul(pj[:], pwb[:], eTb[b][:],
                                 start=True, stop=True)
                nc.vector.scalar_tensor_tensor(hTb[b][:], pj[:],
                                               wtiles["pb"][:, 0:1],
                                               hTb[b][:],
                                               op0=A.add, op1=A.add)
                y2 = pot.tile([64, 256], DT, tag="y2")
                layer_norm(y2[:], hTb[b][:], wtiles["l2g"], wtiles["l2b"],
                           256, pps, pot)
                f1a = pps.tile([128, 256], DT, tag="f1a")
                f1b = pps.tile([128, 256], DT, tag="f1b")
                nc.tensor.matmul(f1a[:], wtiles["w1"][:, 0:128], y2[:],
                                 start=True, stop=True)
                nc.tensor.matmul(f1b[:], wtiles["w1"][:, 128:256], y2[:],
                                 start=True, stop=True)
                r1a = pot.tile([128, 256], DT, tag="r1a")
                r1b = pot.tile([128, 256], DT, tag="r1b")
                nc.scalar.activation(r1a[:], f1a[:], AF.Relu,
                                     bias=wtiles["b1h"][:, 0:1])
                nc.scalar.activation(r1b[:], f1b[:], AF.Relu,
                                     bias=wtiles["b1h"][:, 1:2])
                f2 = pps.tile([64, 256], DT, tag="f2")
                nc.tensor.matmul(f2[:], wtiles["w2a"][:], r1a[:],
                                 start=True, stop=False)
                nc.tensor.matmul(f2[:], wtiles["w2b"][:], r1b[:],
                                 start=False, stop=True)
                nc.vector.scalar_tensor_tensor(hTb[b][:], f2[:],
                                               wtiles["b2"][:, 0:1],
                                               hTb[b][:],
                                               op0=A.add, op1=A.add)
                # token-major + stage into cc3i; per-batch AllGather
                for ah in range(2):
                    tp = pps.tile([128, 64], DT, tag="tok")
                    nc.tensor.transpose(
                        tp[:], hTb[b][:, ah * 128:(ah + 1) * 128], eye64[:])
                    tsb = pot.tile([128, 64], DT, tag="toksb")
                    nc.scalar.copy(tsb[:], tp[:])
                    dstv = cc3i[b][0:8,
                                   ah * 1024:ah * 1024 + 1024].rearrange(
                        "i (a s) -> a i s", a=16)
                    nc.sync.dma_start(dstv, tsb[:])
                nc.gpsimd.collective_compute(
                    "AllGather", A.bypass, ins=[cc3i[b][:]],
                    outs=[cc3o[b][:]], replica_groups=RG)

        # ======== Booster
        with nc.named_scope("booster"), \
             tc.tile_pool(name="hrb", bufs=1) as hrp, \
             tc.tile_pool(name="bprod", bufs=2) as bpp:
            bwt = bwp.tile([128, T4], DT, tag="bwt")
            nc.sync.dma_start(
                bwt[:], tn["bw"][:].rearrange("(m p) t -> p m t", m=4))
            hrbt = []
            for b in range(B):
                t = hrp.tile([128, T], DT, tag=f"hrb{b}", name=f"hrb{b}")
                src = cc3o[b][:].rearrange("r (j t) -> r j t", j=8)
                nc.sync.dma_start(t[0:64, :], src)
                nc.sync.dma_start(t[64:128, :], t[0:64, :])
                hrbt.append(t)
            for b in range(B):
                for m in range(4):
                    msl = slice(m * T, (m + 1) * T)
                    pr = bpp.tile([128, T], DT, tag="bprod", bufs=4)
                    if m == 0:
                        nc.gpsimd.tensor_mul(pr[:], bwt[:, msl], hrbt[b][:])
                    else:
                        nc.vector.tensor_mul(pr[:], bwt[:, msl], hrbt[b][:])
                    r0 = (b * 8 + 2 * m) * 64
                    nc.sync.dma_start(out[r0:r0 + 128, :], pr[:])


def _prep_host(inputs):
    """Host-side prep: shard x/fw/bw per core; pack small weights."""
    f32 = np.float32
    g = {k: np.asarray(v, dtype=f32) for k, v in inputs.items()}
    x = g["x"].reshape(B, SD, SD, T)          # flat view (b, i, j, t')
    fw, bw = g["feebler_w"], g["booster_w"]
    wq, wk, wv = g["wq"], g["wk"], g["wv"]
    wqkv = np.concatenate([w.transpose(1, 0, 2).reshape(SD, SD)
                           for w in (wq, wk, wv)], axis=1)  # [64, 192]
    shared = {
        "wqkv": np.ascontiguousarray(wqkv),
        "pw": np.ascontiguousarray(g["proj_w"]),
        "pb": g["proj_b"].reshape(SD, 1).copy(),
        "l1g": g["ln1_g"].reshape(SD, 1).copy(),
        "l1b": g["ln1_b"].reshape(SD, 1).copy(),
        "l2g": g["ln2_g"].reshape(SD, 1).copy(),
        "l2b": g["ln2_b"].reshape(SD, 1).copy(),
        "w1": np.ascontiguousarray(g["w1"]),
        "b1h": np.ascontiguousarray(g["b1"].reshape(2, 128).T),
        "w2": np.ascontiguousarray(g["w2"]),
        "b2": g["b2"].reshape(SD, 1).copy(),
        "eye64": np.eye(64, dtype=f32),
    }
    in_maps = []
    for k in range(N_CORES):
        i0 = k * IPC
        m = dict(shared)
        m["x"] = np.ascontiguousarray(
            x[:, i0:i0 + IPC].reshape(B * IPC * SD, T))
        m["fw"] = np.ascontiguousarray(
            fw[i0:i0 + IPC].reshape(IPC * SD, T))
        m["bw"] = np.ascontiguousarray(
            bw[i0:i0 + IPC].reshape(IPC * SD, T))
        in_maps.append(m)
    return in_maps


def _get_nc():
    if "nc" not in _CACHE:
        _CACHE["nc"] = _build_nc()
    return _CACHE["nc"]


def run(inputs, trace=False, **kw):
    nc = _get_nc()
    in_maps = _prep_host(inputs)
    res = run_bass_kernel_spmd(nc, in_maps, core_ids=list(range(N_CORES)),
                               trace=trace, **kw)
    full = np.empty((B, SD, SD, T), dtype=np.float32)
    for k in range(N_CORES):
        i0 = k * IPC
        full[:, i0:i0 + IPC] = res.results[k]["out"].reshape(B, IPC, SD, T)
    return full.reshape(B, T, NE), res


def kernel(**inputs):
    out, _ = run(inputs)
    return out



# revision 23
# speedup vs baseline: 1.4784x; 1.4784x over previous
"""Trainium2 Bass kernel for nn_Block_69191923139027 (dense_transformer).

Sharding: 8 cores; core k owns Feebler/Booster rows i in [8k, 8k+8), which
is exactly tokens [256k, 256k+256) per batch, so the Booster needs only
locally-computed h (no h AllGather). Two tiny AllGathers stitch the global
k/v sums and softmax denominators.

v4: feebler j-reduce via ones2-stationary matmuls on fp16 products
(x/fw host-cast to fp16), h -> token-major via PE transposes, batched
LN/FFN over all 1024 local tokens, booster broadcast via SEL-matrix
matmuls on fp16 h, fp16 bw and fp16 output (host casts to fp32).

Self-contained: hardcodes all shapes; no sibling imports.
"""

import numpy as np

import concourse.bacc as bacc
import concourse.mybir as mybir
import concourse.tile as tile
from concourse.bass_utils import run_bass_kernel_spmd

N_CORES = 8
B, T, SD, NE = 4, 2048, 64, 4096
H, HS, FH = 8, 8, 256
EPS = 1e-5
IPC = SD // N_CORES          # 8 feebler rows per core
TLOC = B * IPC * 32          # 1024 local tokens; hT col = b*256 + i*32 + a
DT = mybir.dt.float32
F16 = mybir.dt.float16
RG = [list(range(N_CORES))]
ESHIFT = 64.0                # softmax logit shift (max |logit| ~ 71)

_CACHE = {}


def _build_nc():
    nc = bacc.Bacc("TRN2", target_bir_lowering=False, debug=False,
                   num_devices=N_CORES)
    A = mybir.AluOpType
    AF = mybir.ActivationFunctionType

    tn = {}
    tn["x"] = nc.dram_tensor("x", [B * IPC * SD, T], F16, kind="ExternalInput")
    tn["fw"] = nc.dram_tensor("fw", [IPC * SD, T], F16, kind="ExternalInput")
    tn["bw"] = nc.dram_tensor("bw", [IPC * SD, T], F16, kind="ExternalInput")
    tn["wqkv"] = nc.dram_tensor("wqkv", [SD, 3 * SD], DT, kind="ExternalInput")
    tn["pw"] = nc.dram_tensor("pw", [SD, SD], DT, kind="ExternalInput")
    tn["pb"] = nc.dram_tensor("pb", [SD, 1], DT, kind="ExternalInput")
    tn["l1g"] = nc.dram_tensor("l1g", [SD, 1], DT, kind="ExternalInput")
    tn["l1b"] = nc.dram_tensor("l1b", [SD, 1], DT, kind="ExternalInput")
    tn["l2g"] = nc.dram_tensor("l2g", [SD, 1], DT, kind="ExternalInput")
    tn["l2b"] = nc.dram_tensor("l2b", [SD, 1], DT, kind="ExternalInput")
    tn["w1"] = nc.dram_tensor("w1", [SD, FH], DT, kind="ExternalInput")
    tn["b1h"] = nc.dram_tensor("b1h", [128, 2], DT, kind="ExternalInput")
    tn["w2"] = nc.dram_tensor("w2", [FH, SD], DT, kind="ExternalInput")
    tn["b2"] = nc.dram_tensor("b2", [SD, 1], DT, kind="ExternalInput")
    tn["eye64"] = nc.dram_tensor("eye64", [64, 64], DT, kind="ExternalInput")
    tn["selh"] = nc.dram_tensor("selh", [32, 2048], F16, kind="ExternalInput")
    tn["ones32"] = nc.dram_tensor("ones32", [128, 512], F16,
                                  kind="ExternalInput")
    out = nc.dram_tensor("out", [B * IPC * SD, T], F16, kind="ExternalOutput")
    if _CACHE.get("debug"):
        tn["dbg_h"] = nc.dram_tensor("dbg_h", [32, T], DT,
                                     kind="ExternalOutput")
        tn["dbg_hT"] = nc.dram_tensor("dbg_hT", [64, TLOC], DT,
                                      kind="ExternalOutput")
        tn["dbg_h2h"] = nc.dram_tensor("dbg_h2h", [32, T], F16,
                                       kind="ExternalOutput")
        tn["dbg_bch"] = nc.dram_tensor("dbg_bch", [128, T], F16,
                                       kind="ExternalOutput")
        tn["dbg_pr"] = nc.dram_tensor("dbg_pr", [128, T], F16,
                                      kind="ExternalOutput")
        tn["dbg_bwt"] = nc.dram_tensor("dbg_bwt", [128, T], F16,
                                       kind="ExternalOutput")

    with tile.TileContext(nc) as tc:
        _body(nc, tc, tn, out, A, AF)
    nc.compile()
    return nc


def _body(nc, tc, tn, out, A, AF):
    X = mybir.AxisListType.X

    with tc.tile_pool(name="wconst", bufs=1) as wp, \
         tc.tile_pool(name="mid", bufs=1) as mp, \
         tc.tile_pool(name="bwpool", bufs=1) as bwp, \
         tc.tile_pool(name="dram", bufs=1, space="DRAM") as dp:

        # ---- on-chip constants ----
        # ones32[:, (b*4+m)*32 : +32]: reduction matrix mapping the
        # (i-parity, j) partition sums of chunk (b, m) onto h_sb rows
        # b*8+2m / b*8+2m+1 of a 32-row accumulating PSUM tile.
        ones32 = wp.tile([128, 512], F16, tag="ones32")
        nc.sync.dma_start(ones32[:], tn["ones32"][:])
        ones64 = wp.tile([SD, 1], DT, tag="ones64")
        nc.vector.memset(ones64[:], 1.0 / SD)
        epsv = wp.tile([1, 1], DT, tag="epsv")
        nc.vector.memset(epsv[:], EPS)
        neg64 = wp.tile([64, 1], DT, tag="neg64")
        nc.vector.memset(neg64[:], -ESHIFT)
        # SEL matrices for booster row-broadcast: SEL[:, (b*4+m)*128:+128]
        # selects h2 rows b*8+2m (out rows 0:64) and b*8+2m+1 (64:128).
        selh = wp.tile([32, 2048], F16, tag="selh")
        nc.sync.dma_start(selh[:], tn["selh"][:])

        # ---- kick the collectives bootstrap barrier early ----
        dumi = dp.tile([64, 1], DT, tag="dumi")
        dumo = dp.tile([512, 1], DT, tag="dumo", addr_space="Shared")
        nc.sync.dma_start(dumi[:], ones64[:])
        nc.gpsimd.collective_compute("AllGather", A.bypass, ins=[dumi[:]],
                                     outs=[dumo[:]], replica_groups=RG)

        # ---- small weights ----
        wtiles = {}

        def wtile(name, shape, src, dtype=DT):
            t = wp.tile(shape, dtype, tag=name, name=f"w_{name}")
            nc.sync.dma_start(t[:], src)
            wtiles[name] = t
            return t

        wtile("wqkv", [SD, 3 * SD], tn["wqkv"][:])
        wtile("pw", [SD, SD], tn["pw"][:])
        wtile("pb", [SD, 1], tn["pb"][:])
        wtile("l1g", [SD, 1], tn["l1g"][:])
        wtile("l1b", [SD, 1], tn["l1b"][:])
        wtile("l2g", [SD, 1], tn["l2g"][:])
        wtile("l2b", [SD, 1], tn["l2b"][:])
        wtile("w1", [SD, FH], tn["w1"][:])
        wtile("b1h", [128, 2], tn["b1h"][:])
        wtile("w2a", [128, SD], tn["w2"][0:128, :])
        wtile("w2b", [128, SD], tn["w2"][128:256, :])
        wtile("b2", [SD, 1], tn["b2"][:])
        eye64 = wtile("eye64", [64, 64], tn["eye64"][:])

        # ---- persistent mid-size tiles ----
        h_sb = mp.tile([32, T], DT, tag="h_sb")       # row b*8+i, col a*64+s
        hT = mp.tile([64, TLOC], DT, tag="hT")        # row s, col b*256+i*32+a
        y1 = mp.tile([64, TLOC], DT, tag="y1")
        eT = mp.tile([64, TLOC], DT, tag="eT")
        part = mp.tile([64, B], DT, tag="part")
        zp = mp.tile([64, B], DT, tag="zp")
        sc = mp.tile([64, B], DT, tag="sc")
        h2h = mp.tile([32, T], F16, tag="h2h")        # final h, i-major

        hT4 = hT[:].rearrange("s (b i a) -> s b i a", b=B, i=IPC)

        # layer norm over channel dim s (partitions) for W token columns.
        def layer_norm(y_out, h_ap, g, bta, W, pls, lnp):
            sq = lnp.tile([64, W], DT, tag="ln_sq")
            nc.vector.tensor_mul(sq[:], h_ap, h_ap)
            stats_ps = pls.tile([1, 2 * W], DT, tag="ln_stats")
            for c in range(0, W, 512):
                sl = slice(c, min(c + 512, W))
                slm = slice(W + c, W + min(c + 512, W))
                nc.tensor.matmul(stats_ps[:, sl], ones64[:], h_ap[:, sl],
                                 start=True, stop=True)
                nc.tensor.matmul(stats_ps[:, slm], ones64[:], sq[:, sl],
                                 start=True, stop=True)
            mean_ps = stats_ps[:, 0:W]
            msq_ps = stats_ps[:, W:2 * W]
            mean_sb = lnp.tile([1, W], DT, tag="ln_mean_sb")
            nc.scalar.copy(mean_sb[:], mean_ps)
            mbsq = lnp.tile([1, W], DT, tag="ln_mbsq")
            nc.vector.tensor_mul(mbsq[:], mean_sb[:], mean_sb[:])
            var = lnp.tile([1, W], DT, tag="ln_var")
            nc.vector.tensor_sub(var[:], msq_ps, mbsq[:])
            nc.vector.tensor_scalar_add(var[:], var[:], epsv[0:1, 0:1])
            rvar = lnp.tile([1, W], DT, tag="ln_rvar")
            nc.vector.reciprocal(rvar[:], var[:])
            rstd = lnp.tile([1, W], DT, tag="ln_rstd")
            nc.scalar.sqrt(rstd[:], rvar[:])
            meanb = lnp.tile([64, W], DT, tag="ln_meanb")
            nc.gpsimd.partition_broadcast(meanb[:], mean_sb[:])
            rstdb = lnp.tile([64, W], DT, tag="ln_rstdb")
            nc.gpsimd.partition_broadcast(rstdb[:], rstd[:])
            ymm = lnp.tile([64, W], DT, tag="ln_ymm")
            nc.vector.tensor_sub(ymm[:], h_ap, meanb[:])
            nc.vector.scalar_tensor_tensor(y_out, ymm[:], g[:, 0:1], rstdb[:],
                                           op0=A.mult, op1=A.mult)
            nc.vector.tensor_scalar_add(y_out, y_out, bta[:, 0:1])

        # ======== Phase A: stream x in two t'-halves; feebler + transpose
        with nc.named_scope("feebler"), \
             tc.tile_pool(name="fw", bufs=1) as fwp, \
             tc.tile_pool(name="xin", bufs=2) as xp, \
             tc.tile_pool(name="prod", bufs=2) as prp, \
             tc.tile_pool(name="psA", bufs=1, space="PSUM") as psA:
            fwt = fwp.tile([128, 4 * T], F16, tag="fwt")
            nc.sync.dma_start(
                fwt[:], tn["fw"][:].rearrange("(m p) t -> p m t", m=4))
            fwv = fwt[:].rearrange("p (m t) -> p m t", m=4)
            for c in range(2):          # t' halves of 1024
                csl = slice(c * 1024, (c + 1) * 1024)
                xs = xp.tile([128, 16384], F16, tag="xs")
                for b in range(B):
                    nc.sync.dma_start(
                        xs[:, b * 4096:(b + 1) * 4096],
                        tn["x"][b * 512:(b + 1) * 512, csl].rearrange(
                            "(q p) t -> p q t", q=4))
                h_ps = psA.tile([32, 1024], DT, tag="h_ps", bufs=2)
                for b in range(B):
                    prod = prp.tile([128, 4096], F16, tag="prod")
                    nc.vector.tensor_mul(
                        prod[:].rearrange("p (m t) -> p m t", m=4),
                        xs[:, b * 4096:(b + 1) * 4096].rearrange(
                            "p (m t) -> p m t", m=4),
                        fwv[:, :, csl])
                    for m in range(4):
                        o32 = ones32[:, (b * 4 + m) * 32:
                                     (b * 4 + m + 1) * 32]
                        for u in range(2):
                            nc.tensor.matmul(
                                h_ps[:, u * 512:(u + 1) * 512],
                                o32,
                                prod[:, m * 1024 + u * 512:
                                     m * 1024 + (u + 1) * 512],
                                start=(b == 0 and m == 0),
                                stop=(b == 3 and m == 3))
                nc.scalar.copy(h_sb[:, csl], h_ps[:])
                # transposes for finished 128-col chunks of this half
                # (need all batches' rows -> emit after last batch)
                if True:
                    pass
                for cc in range(8):
                    cidx = c * 8 + cc
                    col = cidx * 128
                    tp = psA.tile([128, 32], DT, tag="tp", bufs=2)
                    nc.tensor.transpose(tp[:], h_sb[:, col:col + 128],
                                        eye64[0:32, 0:32])
                    for da in range(2):
                        nc.scalar.copy(
                            hT4[:, :, :, 2 * cidx + da],
                            tp[da * 64:(da + 1) * 64, :].rearrange(
                                "s (b i) -> s b i", b=B))

        if "dbg_h" in tn:
            nc.sync.dma_start(tn["dbg_h"][:], h_sb[:])
            nc.sync.dma_start(tn["dbg_hT"][:], hT[:])

        # prefetch booster weights into the post-stream DMA gap
        bwt = bwp.tile([128, 4 * T], F16, tag="bwt")
        nc.sync.dma_start(
            bwt[:], tn["bw"][:].rearrange("(m p) t -> p m t", m=4))

        # ======== LN1 -> y1; AG1 (y sums); q; exp; AG2 (denoms)
        cc1i = dp.tile([64, B], DT, tag="cc1i")
        cc1o = dp.tile([512, B], DT, tag="cc1o", addr_space="Shared")
        cc2i = dp.tile([64, B], DT, tag="cc2i")
        cc2o = dp.tile([512, B], DT, tag="cc2o", addr_space="Shared")
        with nc.named_scope("attn"), \
             tc.tile_pool(name="ln1t", bufs=1) as lnp1, \
             tc.tile_pool(name="psL", bufs=1, space="PSUM") as psL:
            layer_norm(y1[:], hT[:], wtiles["l1g"], wtiles["l1b"],
                       TLOC, psL, lnp1)
            nc.vector.tensor_reduce(
                part[:], y1[:].rearrange("s (b t) -> s b t", b=B),
                axis=X, op=A.add)
            nc.sync.dma_start(cc1i[:], part[:])
            nc.gpsimd.collective_compute("AllGather", A.bypass,
                                         ins=[cc1i[:]], outs=[cc1o[:]],
                                         replica_groups=RG)
            # q overlaps the AllGather
            q_ps = psL.tile([64, TLOC], DT, tag="q_ps")
            for u in range(2):
                sl = slice(u * 512, (u + 1) * 512)
                nc.tensor.matmul(q_ps[:, sl], wtiles["wqkv"][:, 0:64],
                                 y1[:, sl], start=True, stop=True)
            gath = lnp1.tile([64, 4 * N_CORES], DT, tag="gath")
            nc.sync.dma_start(
                gath[:].rearrange("p (s r) -> p s r", s=B),
                cc1o[:].rearrange("(r p) s -> p s r", r=N_CORES))
            ysum = lnp1.tile([64, B], DT, tag="ysum")
            nc.vector.tensor_reduce(
                ysum[:], gath[:].rearrange("p (s r) -> p s r", s=B),
                axis=X, op=A.add)
            kv_ps = psL.tile([128, B], DT, tag="kv_ps")
            nc.tensor.matmul(kv_ps[:], wtiles["wqkv"][:, 64:192], ysum[:],
                             start=True, stop=True)
            kvg = lnp1.tile([128, B], DT, tag="kvg")
            nc.scalar.copy(kvg[:], kv_ps[:])
            for b in range(B):
                sl = slice(b * 256, (b + 1) * 256)
                nc.scalar.activation(eT[:, sl], q_ps[:, sl], AF.Exp,
                                     bias=neg64[:, 0:1],
                                     scale=kvg[0:64, b:b + 1],
                                     accum_out=zp[:, b:b + 1])
            nc.sync.dma_start(cc2i[:], zp[:])
            nc.gpsimd.collective_compute("AllGather", A.bypass,
                                         ins=[cc2i[:]], outs=[cc2o[:]],
                                         replica_groups=RG)
            gath2 = lnp1.tile([64, 4 * N_CORES], DT, tag="gath2")
            nc.sync.dma_start(
                gath2[:].rearrange("p (s r) -> p s r", s=B),
                cc2o[:].rearrange("(r p) s -> p s r", r=N_CORES))
            zg = lnp1.tile([64, B], DT, tag="zg")
            nc.vector.tensor_reduce(
                zg[:], gath2[:].rearrange("p (s r) -> p s r", s=B),
                axis=X, op=A.add)
            rz = lnp1.tile([64, B], DT, tag="rz")
            nc.vector.reciprocal(rz[:], zg[:])
            cvg = lnp1.tile([64, B], DT, tag="cvg")
            nc.scalar.copy(cvg[:], kvg[64:128, :])
            nc.vector.tensor_mul(sc[:], cvg[:], rz[:])

        # ======== proj -> LN2 -> FFN, batched over all 1024 tokens
        with nc.named_scope("midp"), \
             tc.tile_pool(name="pot", bufs=1) as pot:
            with tc.tile_pool(name="psM1", bufs=1, space="PSUM") as pm1:
                pj = pm1.tile([64, TLOC], DT, tag="pj")
                for b in range(B):
                    sl = slice(b * 256, (b + 1) * 256)
                    pwb = pot.tile([64, 64], DT, tag="pwb", bufs=2)
                    nc.vector.tensor_scalar_mul(pwb[:], wtiles["pw"][:],
                                                sc[:, b:b + 1])
                    nc.tensor.matmul(pj[:, sl], pwb[:], eT[:, sl],
                                     start=True, stop=True)
                nc.vector.scalar_tensor_tensor(hT[:], pj[:],
                                               wtiles["pb"][:, 0:1], hT[:],
                                               op0=A.add, op1=A.add)
                y2 = pot.tile([64, TLOC], DT, tag="y2")
                layer_norm(y2[:], hT[:], wtiles["l2g"], wtiles["l2b"],
                           TLOC, pm1, pot)
            with tc.tile_pool(name="psM2", bufs=1, space="PSUM") as pm2:
                f1a = pm2.tile([128, TLOC], DT, tag="f1a")
                f1b = pm2.tile([128, TLOC], DT, tag="f1b")
                for u in range(2):
                    sl = slice(u * 512, (u + 1) * 512)
                    nc.tensor.matmul(f1a[:, sl], wtiles["w1"][:, 0:128],
                                     y2[:, sl], start=True, stop=True)
                    nc.tensor.matmul(f1b[:, sl], wtiles["w1"][:, 128:256],
                                     y2[:, sl], start=True, stop=True)
                r1a = pot.tile([128, TLOC], DT, tag="r1a")
                r1b = pot.tile([128, TLOC], DT, tag="r1b")
                nc.scalar.activation(r1a[:], f1a[:], AF.Relu,
                                     bias=wtiles["b1h"][:, 0:1])
                nc.scalar.activation(r1b[:], f1b[:], AF.Relu,
                                     bias=wtiles["b1h"][:, 1:2])
                f2 = pm2.tile([64, TLOC], DT, tag="f2")
                for u in range(2):
                    sl = slice(u * 512, (u + 1) * 512)
                    nc.tensor.matmul(f2[:, sl], wtiles["w2a"][:],
                                     r1a[:, sl], start=True, stop=False)
                    nc.tensor.matmul(f2[:, sl], wtiles["w2b"][:],
                                     r1b[:, sl], start=False, stop=True)
                nc.vector.scalar_tensor_tensor(hT[:], f2[:],
                                               wtiles["b2"][:, 0:1], hT[:],
                                               op0=A.add, op1=A.add)

        # ======== Booster: back-transpose h to i-major, broadcast rows via
        # SEL matmuls, multiply with bw, stream out. All local.
        with nc.named_scope("booster"), \
             tc.tile_pool(name="bst", bufs=1) as bst, \
             tc.tile_pool(name="psB", bufs=1, space="PSUM") as psB:
            # h2h[(b,i), a*64+s] = hT[s, b*256+i*32+a]: transpose the
            # 32 same-a token columns so (b,i) lands on partitions.
            for a in range(32):
                tpb = psB.tile([32, 64], DT, tag="tpb", bufs=2)
                nc.tensor.transpose(tpb[:], hT4[:, :, :, a], eye64[:])
                nc.scalar.copy(h2h[:, a * 64:(a + 1) * 64], tpb[:])
            if "dbg_h2h" in tn:
                nc.sync.dma_start(tn["dbg_h2h"][:], h2h[:])
            for b in range(B):
                for m in range(4):
                    pr = bst.tile([128, T], F16, tag="pr", bufs=3)
                    for half in range(2):
                        bc = psB.tile([128, 1024], DT, tag="bc", bufs=3)
                        hsl = slice(half * 1024, (half + 1) * 1024)
                        for u in range(2):
                            us = slice(half * 1024 + u * 512,
                                       half * 1024 + (u + 1) * 512)
                            nc.tensor.matmul(
                                bc[:, u * 512:(u + 1) * 512],
                                selh[:, (b * 4 + m) * 128:
                                     (b * 4 + m) * 128 + 128],
                                h2h[:, us], start=True, stop=True)
                        bch = bst.tile([128, 1024], F16, tag="bch", bufs=3)
                        nc.scalar.copy(bch[:], bc[:])
                        nc.vector.tensor_mul(
                            pr[:, hsl], bwt[:, m * T + half * 1024:
                                            m * T + (half + 1) * 1024],
                            bch[:])
                        if b == 0 and m == 0 and "dbg_bch" in tn:
                            nc.sync.dma_start(tn["dbg_bch"][:, hsl], bch[:])
                    r0 = (b * 8 + 2 * m) * 64
                    nc.sync.dma_start(out[r0:r0 + 128, :], pr[:])
                    if b == 0 and m == 0 and "dbg_pr" in tn:
                        nc.sync.dma_start(tn["dbg_pr"][:], pr[:])
                        nc.sync.dma_start(tn["dbg_bwt"][:], bwt[:, 0:T])


def _prep_host(inputs):
    """Host-side prep: shard x/fw/bw per core (fp16); pack small weights."""
    f32 = np.float32
    g = {k: np.asarray(v, dtype=f32) for k, v in inputs.items()}
    x = g["x"].reshape(B, SD, SD, T)          # flat view (b, i, j, t')
    fw, bw = g["feebler_w"], g["booster_w"]
    wq, wk, wv = g["wq"], g["wk"], g["wv"]
    wqkv = np.concatenate([w.transpose(1, 0, 2).reshape(SD, SD)
                           for w in (wq, wk, wv)], axis=1)  # [64, 192]
    shared = {
        "wqkv": np.ascontiguousarray(wqkv),
        "pw": np.ascontiguousarray(g["proj_w"]),
        "pb": g["proj_b"].reshape(SD, 1).copy(),
        "l1g": g["ln1_g"].reshape(SD, 1).copy(),
        "l1b": g["ln1_b"].reshape(SD, 1).copy(),
        "l2g": g["ln2_g"].reshape(SD, 1).copy(),
        "l2b": g["ln2_b"].reshape(SD, 1).copy(),
        "w1": np.ascontiguousarray(g["w1"]),
        "b1h": np.ascontiguousarray(g["b1"].reshape(2, 128).T),
        "w2": np.ascontiguousarray(g["w2"]),
        "b2": g["b2"].reshape(SD, 1).copy(),
        "eye64": np.eye(64, dtype=f32),
    }
    sel = np.zeros((32, 2048), np.float16)
    for b in range(B):
        for m in range(4):
            c0 = (b * 4 + m) * 128
            sel[b * 8 + 2 * m, c0:c0 + 64] = 1.0
            sel[b * 8 + 2 * m + 1, c0 + 64:c0 + 128] = 1.0
    shared["selh"] = sel
    o32 = np.zeros((128, 512), np.float16)
    for b in range(B):
        for m in range(4):
            c0 = (b * 4 + m) * 32
            o32[0:64, c0 + b * 8 + 2 * m] = 1.0
            o32[64:128, c0 + b * 8 + 2 * m + 1] = 1.0
    shared["ones32"] = o32
    in_maps = []
    for k in range(N_CORES):
        i0 = k * IPC
        m = dict(shared)
        m["x"] = np.ascontiguousarray(
            x[:, i0:i0 + IPC].reshape(B * IPC * SD, T)).astype(np.float16)
        m["fw"] = np.ascontiguousarray(
            fw[i0:i0 + IPC].reshape(IPC * SD, T)).astype(np.float16)
        # booster output is sharded over j (rev[b,i,j]=bw[i,j]*hr[b,j]):
        # rows (j_loc, i) so the broadcast h row per 64-row group is local
        m["bw"] = np.ascontiguousarray(
            bw[:, i0:i0 + IPC].transpose(1, 0, 2).reshape(
                IPC * SD, T)).astype(np.float16)
        in_maps.append(m)
    return in_maps


def _get_nc():
    if "nc" not in _CACHE:
        _CACHE["nc"] = _build_nc()
    return _CACHE["nc"]


def run(inputs, trace=False, **kw):
    nc = _get_nc()
    in_maps = _prep_host(inputs)
    res = run_bass_kernel_spmd(nc, in_maps, core_ids=list(range(N_CORES)),
                               trace=trace, **kw)
    full = np.empty((B, SD, SD, T), dtype=np.float32)
    for k in range(N_CORES):
        i0 = k * IPC
        co = res.results[k]["out"].astype(np.float32).reshape(B, IPC, SD, T)
        full[:, :, i0:i0 + IPC] = co.transpose(0, 2, 1, 3)
    return full.reshape(B, T, NE), res


def kernel(**inputs):
    out, _ = run(inputs)
    return out


# revision 32
# speedup vs baseline: 1.6088x; 1.0882x over previous
"""Trainium2 Bass kernel for nn_Block_69191923139027 (dense_transformer).

Sharding: 8 cores; core k owns Feebler/Booster rows i in [8k, 8k+8), which
is exactly tokens [256k, 256k+256) per batch, so the Booster needs only
locally-computed h (no h AllGather). Two tiny AllGathers stitch the global
k/v sums and softmax denominators.

v4: feebler j-reduce via ones2-stationary matmuls on fp16 products
(x/fw host-cast to fp16), h -> token-major via PE transposes, batched
LN/FFN over all 1024 local tokens, booster broadcast via SEL-matrix
matmuls on fp16 h, fp16 bw and fp16 output (host casts to fp32).

Self-contained: hardcodes all shapes; no sibling imports.
"""

import numpy as np

import concourse.bacc as bacc
import concourse.mybir as mybir
import concourse.tile as tile
from concourse.bass_utils import run_bass_kernel_spmd

N_CORES = 8
B, T, SD, NE = 4, 2048, 64, 4096
H, HS, FH = 8, 8, 256
EPS = 1e-5
IPC = SD // N_CORES          # 8 feebler rows per core
TLOC = B * IPC * 32          # 1024 local tokens; hT col = b*256 + i*32 + a
DT = mybir.dt.float32
F16 = mybir.dt.float16
RG = [list(range(N_CORES))]
ESHIFT = 64.0                # softmax logit shift (max |logit| ~ 71)

_CACHE = {}


def _build_nc():
    nc = bacc.Bacc("TRN2", target_bir_lowering=False, debug=False,
                   num_devices=N_CORES)
    A = mybir.AluOpType
    AF = mybir.ActivationFunctionType

    tn = {}
    tn["x"] = nc.dram_tensor("x", [B * IPC * SD, T], F16, kind="ExternalInput")
    tn["fw"] = nc.dram_tensor("fw", [IPC * SD, T], F16, kind="ExternalInput")
    tn["bw"] = nc.dram_tensor("bw", [IPC * SD, T], F16, kind="ExternalInput")
    tn["wqkv"] = nc.dram_tensor("wqkv", [SD, 3 * SD], DT, kind="ExternalInput")
    tn["pw"] = nc.dram_tensor("pw", [SD, SD], DT, kind="ExternalInput")
    tn["pb"] = nc.dram_tensor("pb", [SD, 1], DT, kind="ExternalInput")
    tn["l1g"] = nc.dram_tensor("l1g", [SD, 1], DT, kind="ExternalInput")
    tn["l1b"] = nc.dram_tensor("l1b", [SD, 1], DT, kind="ExternalInput")
    tn["l2g"] = nc.dram_tensor("l2g", [SD, 1], DT, kind="ExternalInput")
    tn["l2b"] = nc.dram_tensor("l2b", [SD, 1], DT, kind="ExternalInput")
    tn["w1"] = nc.dram_tensor("w1", [SD, FH], DT, kind="ExternalInput")
    tn["b1h"] = nc.dram_tensor("b1h", [128, 2], DT, kind="ExternalInput")
    tn["w2"] = nc.dram_tensor("w2", [FH, SD], DT, kind="ExternalInput")
    tn["b2"] = nc.dram_tensor("b2", [SD, 1], DT, kind="ExternalInput")
    tn["eye64"] = nc.dram_tensor("eye64", [64, 64], DT, kind="ExternalInput")
    tn["selh"] = nc.dram_tensor("selh", [32, 2048], F16, kind="ExternalInput")
    tn["ones32"] = nc.dram_tensor("ones32", [128, 512], F16,
                                  kind="ExternalInput")
    out = nc.dram_tensor("out", [B * IPC * SD, T], F16, kind="ExternalOutput")
    if _CACHE.get("debug"):
        tn["dbg_h"] = nc.dram_tensor("dbg_h", [32, T], DT,
                                     kind="ExternalOutput")
        tn["dbg_hT"] = nc.dram_tensor("dbg_hT", [64, TLOC], DT,
                                      kind="ExternalOutput")
        tn["dbg_h2h"] = nc.dram_tensor("dbg_h2h", [32, T], F16,
                                       kind="ExternalOutput")

    with tile.TileContext(nc) as tc:
        _body(nc, tc, tn, out, A, AF)
    nc.compile()
    return nc


def _body(nc, tc, tn, out, A, AF):
    X = mybir.AxisListType.X

    with tc.tile_pool(name="wconst", bufs=1) as wp, \
         tc.tile_pool(name="mid", bufs=1) as mp, \
         tc.tile_pool(name="bwpool", bufs=1) as bwp, \
         tc.tile_pool(name="dram", bufs=1, space="DRAM") as dp:

        # ---- on-chip constants ----
        # ones32[:, (b*4+m)*32 : +32]: reduction matrix mapping the
        # (i-parity, j) partition sums of chunk (b, m) onto h_sb rows
        # b*8+2m / b*8+2m+1 of a 32-row accumulating PSUM tile.
        ones32 = wp.tile([128, 512], F16, tag="ones32")
        nc.sync.dma_start(ones32[:], tn["ones32"][:])
        ones64 = wp.tile([SD, 1], DT, tag="ones64")
        nc.vector.memset(ones64[:], 1.0 / SD)
        ones1r = wp.tile([1, SD], DT, tag="ones1r")
        nc.vector.memset(ones1r[:], 1.0)
        epsv = wp.tile([1, 1], DT, tag="epsv")
        nc.vector.memset(epsv[:], EPS)
        neg64 = wp.tile([64, 1], DT, tag="neg64")
        nc.vector.memset(neg64[:], -ESHIFT)
        selh = wp.tile([32, 2048], F16, tag="selh")

        # ---- kick the collectives bootstrap barrier early; shaped like
        # the first real AllGather to absorb its first-op overhead ----
        dz = wp.tile([64, B], DT, tag="dz")
        nc.vector.memset(dz[:], 0.0)
        dumi = dp.tile([64, B], DT, tag="dumi")
        dumo = dp.tile([512, B], DT, tag="dumo", addr_space="Shared")
        nc.sync.dma_start(dumi[:], dz[:])
        nc.gpsimd.collective_compute("AllGather", A.bypass, ins=[dumi[:]],
                                     outs=[dumo[:]], replica_groups=RG)

        # ---- small weights (DMAs emitted inside the feebler scope so the
        # x/fw stream goes first) ----
        wtiles = {}

        def wtile(name, shape, src, dtype=DT):
            t = wp.tile(shape, dtype, tag=name, name=f"w_{name}")
            nc.sync.dma_start(t[:], src)
            wtiles[name] = t
            return t

        def emit_small_weights():
            wtile("wqkv", [SD, 3 * SD], tn["wqkv"][:])
            wtile("pw", [SD, SD], tn["pw"][:])
            wtile("pb", [SD, 1], tn["pb"][:])
            wtile("l1g", [SD, 1], tn["l1g"][:])
            wtile("l1b", [SD, 1], tn["l1b"][:])
            wtile("l2g", [SD, 1], tn["l2g"][:])
            wtile("l2b", [SD, 1], tn["l2b"][:])
            wtile("w1", [SD, FH], tn["w1"][:])
            wtile("b1h", [128, 2], tn["b1h"][:])
            wtile("w2a", [128, SD], tn["w2"][0:128, :])
            wtile("w2b", [128, SD], tn["w2"][128:256, :])
            wtile("b2", [SD, 1], tn["b2"][:])
            wtile("eye64", [64, 64], tn["eye64"][:])

        # ---- persistent mid-size tiles ----
        h_sb = mp.tile([32, T], DT, tag="h_sb")       # row b*8+i, col a*64+s
        hT = mp.tile([64, TLOC], DT, tag="hT")        # row s, col b*256+i*32+a
        y1 = mp.tile([64, TLOC], DT, tag="y1")
        eT = mp.tile([64, TLOC], DT, tag="eT")
        part = mp.tile([64, B], DT, tag="part")
        zp = mp.tile([64, B], DT, tag="zp")
        sc = mp.tile([64, B], DT, tag="sc")
        h2h = mp.tile([32, T], F16, tag="h2h")        # final h, i-major

        hT4 = hT[:].rearrange("s (b i a) -> s b i a", b=B, i=IPC)

        # layer norm over channel dim s (partitions) for W token columns.
        # Own scoped PSUM pools; broadcasts via K=1 matmuls on the PE.
        def layer_norm(y_out, h_ap, g, bta, W, lnp, tag):
            sq = lnp.tile([64, W], DT, tag="ln_sq")
            nc.vector.tensor_mul(sq[:], h_ap, h_ap)
            mean_sb = lnp.tile([1, W], DT, tag="ln_mean_sb")
            rstd = lnp.tile([1, W], DT, tag="ln_rstd")
            with tc.tile_pool(name=f"ps_{tag}a", bufs=1, space="PSUM") as pa:
                stats_ps = pa.tile([1, 2 * W], DT, tag="ln_stats")
                for c in range(0, W, 512):
                    sl = slice(c, min(c + 512, W))
                    slm = slice(W + c, W + min(c + 512, W))
                    nc.tensor.matmul(stats_ps[:, sl], ones64[:], h_ap[:, sl],
                                     start=True, stop=True)
                    nc.tensor.matmul(stats_ps[:, slm], ones64[:], sq[:, sl],
                                     start=True, stop=True)
                mean_ps = stats_ps[:, 0:W]
                msq_ps = stats_ps[:, W:2 * W]
                nc.scalar.copy(mean_sb[:], mean_ps)
                mbsq = lnp.tile([1, W], DT, tag="ln_mbsq")
                nc.vector.tensor_mul(mbsq[:], mean_sb[:], mean_sb[:])
                var = lnp.tile([1, W], DT, tag="ln_var")
                nc.vector.tensor_sub(var[:], msq_ps, mbsq[:])
                nc.vector.tensor_scalar_add(var[:], var[:], epsv[0:1, 0:1])
                rvar = lnp.tile([1, W], DT, tag="ln_rvar")
                nc.vector.reciprocal_approx_fast(rvar[:], var[:])
                nc.scalar.sqrt(rstd[:], rvar[:])
            with tc.tile_pool(name=f"ps_{tag}b", bufs=1, space="PSUM") as pb:
                meanb = pb.tile([64, W], DT, tag="ln_meanb")
                rstdb = pb.tile([64, W], DT, tag="ln_rstdb")
                for c in range(0, W, 512):
                    sl = slice(c, min(c + 512, W))
                    nc.tensor.matmul(meanb[:, sl], ones1r[:],
                                     mean_sb[:, sl], start=True, stop=True)
                    nc.tensor.matmul(rstdb[:, sl], ones1r[:],
                                     rstd[:, sl], start=True, stop=True)
                ymm = lnp.tile([64, W], DT, tag="ln_ymm")
                nc.vector.tensor_sub(ymm[:], h_ap, meanb[:])
                nc.vector.scalar_tensor_tensor(y_out, ymm[:], g[:, 0:1],
                                               rstdb[:], op0=A.mult,
                                               op1=A.mult)
                nc.vector.tensor_scalar_add(y_out, y_out, bta[:, 0:1])

        # ======== Phase A: stream x in two t'-halves; feebler + transpose
        with nc.named_scope("feebler"), \
             tc.tile_pool(name="fw", bufs=1) as fwp, \
             tc.tile_pool(name="xin", bufs=2) as xp, \
             tc.tile_pool(name="prod", bufs=2) as prp, \
             tc.tile_pool(name="psA", bufs=1, space="PSUM") as psA:
            fwt = fwp.tile([128, 4 * T], F16, tag="fwt")
            nc.sync.dma_start(
                fwt[:], tn["fw"][:].rearrange("(m p) t -> p m t", m=4))
            fwv = fwt[:].rearrange("p (m t) -> p m t", m=4)
            for c in range(2):          # t' halves of 1024
                csl = slice(c * 1024, (c + 1) * 1024)
                xs = xp.tile([128, 16384], F16, tag="xs")
                for b in range(B):
                    nc.sync.dma_start(
                        xs[:, b * 4096:(b + 1) * 4096],
                        tn["x"][b * 512:(b + 1) * 512, csl].rearrange(
                            "(q p) t -> p q t", q=4))
                if c == 0:
                    emit_small_weights()
                h_ps = psA.tile([32, 1024], DT, tag="h_ps", bufs=2)
                for b in range(B):
                    prod = prp.tile([128, 4096], F16, tag="prod")
                    nc.vector.tensor_mul(
                        prod[:].rearrange("p (m t) -> p m t", m=4),
                        xs[:, b * 4096:(b + 1) * 4096].rearrange(
                            "p (m t) -> p m t", m=4),
                        fwv[:, :, csl])
                    for m in range(4):
                        o32 = ones32[:, (b * 4 + m) * 32:
                                     (b * 4 + m + 1) * 32]
                        for u in range(2):
                            nc.tensor.matmul(
                                h_ps[:, u * 512:(u + 1) * 512],
                                o32,
                                prod[:, m * 1024 + u * 512:
                                     m * 1024 + (u + 1) * 512],
                                start=(b == 0 and m == 0),
                                stop=(b == 3 and m == 3))
                nc.scalar.copy(h_sb[:, csl], h_ps[:])
                # transposes for finished 128-col chunks of this half
                for cc in range(8):
                    cidx = c * 8 + cc
                    col = cidx * 128
                    tp = psA.tile([128, 32], DT, tag="tp", bufs=2)
                    nc.tensor.transpose(tp[:], h_sb[:, col:col + 128],
                                        wtiles["eye64"][0:32, 0:32])
                    for da in range(2):
                        nc.scalar.copy(
                            hT4[:, :, :, 2 * cidx + da],
                            tp[da * 64:(da + 1) * 64, :].rearrange(
                                "s (b i) -> s b i", b=B))

        if "dbg_h" in tn:
            nc.sync.dma_start(tn["dbg_h"][:], h_sb[:])
            nc.sync.dma_start(tn["dbg_hT"][:], hT[:])

        # prefetch booster weights into the post-stream DMA gap
        bwt = bwp.tile([128, 4 * T], F16, tag="bwt")
        nc.sync.dma_start(
            bwt[:], tn["bw"][:].rearrange("(m p) t -> p m t", m=4))

        # ======== LN1 -> y1; AG1 (y sums); q; exp; AG2 (denoms)
        cc1i = dp.tile([64, B], DT, tag="cc1i")
        cc1o = dp.tile([512, B], DT, tag="cc1o", addr_space="Shared")
        cc2i = dp.tile([64, B], DT, tag="cc2i")
        cc2o = dp.tile([512, B], DT, tag="cc2o", addr_space="Shared")
        with nc.named_scope("attn"), \
             tc.tile_pool(name="ln1t", bufs=1) as lnp1:
            layer_norm(y1[:], hT[:], wtiles["l1g"], wtiles["l1b"],
                       TLOC, lnp1, "l1")
            nc.vector.tensor_reduce(
                part[:], y1[:].rearrange("s (b t) -> s b t", b=B),
                axis=X, op=A.add)
            nc.sync.dma_start(cc1i[:], part[:])
            nc.gpsimd.collective_compute("AllGather", A.bypass,
                                         ins=[cc1i[:]], outs=[cc1o[:]],
                                         replica_groups=RG)
            with tc.tile_pool(name="psL", bufs=1, space="PSUM") as psL:
                # q overlaps the AllGather
                q_ps = psL.tile([64, TLOC], DT, tag="q_ps")
                for u in range(2):
                    sl = slice(u * 512, (u + 1) * 512)
                    nc.tensor.matmul(q_ps[:, sl], wtiles["wqkv"][:, 0:64],
                                     y1[:, sl], start=True, stop=True)
                gath = lnp1.tile([64, 4 * N_CORES], DT, tag="gath")
                nc.sync.dma_start(
                    gath[:].rearrange("p (s r) -> p s r", s=B),
                    cc1o[:].rearrange("(r p) s -> p s r", r=N_CORES))
                ysum = lnp1.tile([64, B], DT, tag="ysum")
                nc.vector.tensor_reduce(
                    ysum[:], gath[:].rearrange("p (s r) -> p s r", s=B),
                    axis=X, op=A.add)
                kv_ps = psL.tile([128, B], DT, tag="kv_ps")
                nc.tensor.matmul(kv_ps[:], wtiles["wqkv"][:, 64:192],
                                 ysum[:], start=True, stop=True)
                kvg = lnp1.tile([128, B], DT, tag="kvg")
                nc.scalar.copy(kvg[:], kv_ps[:])
                for b in range(B):
                    sl = slice(b * 256, (b + 1) * 256)
                    nc.scalar.activation(eT[:, sl], q_ps[:, sl], AF.Exp,
                                         bias=neg64[:, 0:1],
                                         scale=kvg[0:64, b:b + 1],
                                         accum_out=zp[:, b:b + 1])
                nc.sync.dma_start(cc2i[:], zp[:])
                nc.gpsimd.collective_compute("AllGather", A.bypass,
                                             ins=[cc2i[:]], outs=[cc2o[:]],
                                             replica_groups=RG)
                gath2 = lnp1.tile([64, 4 * N_CORES], DT, tag="gath2")
                nc.sync.dma_start(
                    gath2[:].rearrange("p (s r) -> p s r", s=B),
                    cc2o[:].rearrange("(r p) s -> p s r", r=N_CORES))
                zg = lnp1.tile([64, B], DT, tag="zg")
                nc.vector.tensor_reduce(
                    zg[:], gath2[:].rearrange("p (s r) -> p s r", s=B),
                    axis=X, op=A.add)
                rz = lnp1.tile([64, B], DT, tag="rz")
                nc.vector.reciprocal(rz[:], zg[:])
                cvg = lnp1.tile([64, B], DT, tag="cvg")
                nc.scalar.copy(cvg[:], kvg[64:128, :])
                nc.vector.tensor_mul(sc[:], cvg[:], rz[:])

        # ======== proj -> LN2 -> FFN, batched over all 1024 tokens
        with nc.named_scope("midp"), \
             tc.tile_pool(name="pot", bufs=1) as pot:
            with tc.tile_pool(name="psM1", bufs=1, space="PSUM") as pm1:
                pj = pm1.tile([64, TLOC], DT, tag="pj")
                for b in range(B):
                    sl = slice(b * 256, (b + 1) * 256)
                    pwb = pot.tile([64, 64], DT, tag="pwb", bufs=2)
                    nc.vector.tensor_scalar_mul(pwb[:], wtiles["pw"][:],
                                                sc[:, b:b + 1])
                    nc.tensor.matmul(pj[:, sl], pwb[:], eT[:, sl],
                                     start=True, stop=True)
                nc.vector.scalar_tensor_tensor(hT[:], pj[:],
                                               wtiles["pb"][:, 0:1], hT[:],
                                               op0=A.add, op1=A.add)
                y2 = pot.tile([64, TLOC], DT, tag="y2")
            layer_norm(y2[:], hT[:], wtiles["l2g"], wtiles["l2b"],
                       TLOC, pot, "l2")
            with tc.tile_pool(name="psM2", bufs=1, space="PSUM") as pm2:
                f1a = pm2.tile([128, TLOC], DT, tag="f1a")
                f1b = pm2.tile([128, TLOC], DT, tag="f1b")
                for u in range(2):
                    sl = slice(u * 512, (u + 1) * 512)
                    nc.tensor.matmul(f1a[:, sl], wtiles["w1"][:, 0:128],
                                     y2[:, sl], start=True, stop=True)
                    nc.tensor.matmul(f1b[:, sl], wtiles["w1"][:, 128:256],
                                     y2[:, sl], start=True, stop=True)
                r1a = pot.tile([128, TLOC], DT, tag="r1a")
                r1b = pot.tile([128, TLOC], DT, tag="r1b")
                nc.scalar.activation(r1a[:], f1a[:], AF.Relu,
                                     bias=wtiles["b1h"][:, 0:1])
                nc.scalar.activation(r1b[:], f1b[:], AF.Relu,
                                     bias=wtiles["b1h"][:, 1:2])
                f2 = pm2.tile([64, TLOC], DT, tag="f2")
                for u in range(2):
                    sl = slice(u * 512, (u + 1) * 512)
                    nc.tensor.matmul(f2[:, sl], wtiles["w2a"][:],
                                     r1a[:, sl], start=True, stop=False)
                    nc.tensor.matmul(f2[:, sl], wtiles["w2b"][:],
                                     r1b[:, sl], start=False, stop=True)
                nc.vector.scalar_tensor_tensor(hT[:], f2[:],
                                               wtiles["b2"][:, 0:1], hT[:],
                                               op0=A.add, op1=A.add)

        # ======== Booster: back-transpose h to local-row-major (DRAM fold),
        # broadcast rows via SEL matmuls, multiply with bw, stream out.
        hr_d = dp.tile([32, T], F16, tag="hr_d")
        with nc.named_scope("booster"), \
             tc.tile_pool(name="bst", bufs=1) as bst, \
             tc.tile_pool(name="psB", bufs=1, space="PSUM") as psB:
            nc.sync.dma_start(selh[:], tn["selh"][:])
            eye64 = wtiles["eye64"]
            for b in range(B):
                for cq in range(2):
                    tpb = psB.tile([128, 64], DT, tag="tpb", bufs=2)
                    col = b * 256 + cq * 128
                    nc.tensor.transpose(tpb[:], hT[:, col:col + 128],
                                        eye64[:])
                    stage = bst.tile([128, 64], F16, tag="stage", bufs=2)
                    nc.scalar.copy(stage[:], tpb[:])
                    r0 = b * 8 + cq * 4
                    nc.sync.dma_start(
                        hr_d[r0:r0 + 4, :].rearrange("i (a s) -> (i a) s",
                                                     a=32),
                        stage[:])
            nc.sync.dma_start(h2h[:], hr_d[:])
            if "dbg_h2h" in tn:
                nc.sync.dma_start(tn["dbg_h2h"][:], h2h[:])
            for b in range(B):
                for m in range(4):
                    pr = bst.tile([128, T], F16, tag="pr", bufs=3)
                    for half in range(2):
                        bc = psB.tile([128, 1024], DT, tag="bc", bufs=3)
                        hsl = slice(half * 1024, (half + 1) * 1024)
                        for u in range(2):
                            us = slice(half * 1024 + u * 512,
                                       half * 1024 + (u + 1) * 512)
                            nc.tensor.matmul(
                                bc[:, u * 512:(u + 1) * 512],
                                selh[:, (b * 4 + m) * 128:
                                     (b * 4 + m) * 128 + 128],
                                h2h[:, us], start=True, stop=True)
                        nc.vector.tensor_mul(
                            pr[:, hsl], bwt[:, m * T + half * 1024:
                                            m * T + (half + 1) * 1024],
                            bc[:])
                    r0 = (b * 8 + 2 * m) * 64
                    nc.sync.dma_start(out[r0:r0 + 128, :], pr[:])


def _prep_host(inputs):
    """Host-side prep: shard x/fw/bw per core (fp16); pack small weights."""
    f32 = np.float32
    g = {k: np.asarray(v, dtype=f32) for k, v in inputs.items()}
    x = g["x"].reshape(B, SD, SD, T)          # flat view (b, i, j, t')
    fw, bw = g["feebler_w"], g["booster_w"]
    wq, wk, wv = g["wq"], g["wk"], g["wv"]
    wqkv = np.concatenate([w.transpose(1, 0, 2).reshape(SD, SD)
                           for w in (wq, wk, wv)], axis=1)  # [64, 192]
    shared = {
        "wqkv": np.ascontiguousarray(wqkv),
        "pw": np.ascontiguousarray(g["proj_w"]),
        "pb": g["proj_b"].reshape(SD, 1).copy(),
        "l1g": g["ln1_g"].reshape(SD, 1).copy(),
        "l1b": g["ln1_b"].reshape(SD, 1).copy(),
        "l2g": g["ln2_g"].reshape(SD, 1).copy(),
        "l2b": g["ln2_b"].reshape(SD, 1).copy(),
        "w1": np.ascontiguousarray(g["w1"]),
        "b1h": np.ascontiguousarray(g["b1"].reshape(2, 128).T),
        "w2": np.ascontiguousarray(g["w2"]),
        "b2": g["b2"].reshape(SD, 1).copy(),
        "eye64": np.eye(64, dtype=f32),
    }
    sel = np.zeros((32, 2048), np.float16)
    for b in range(B):
        for m in range(4):
            c0 = (b * 4 + m) * 128
            sel[b * 8 + 2 * m, c0:c0 + 64] = 1.0
            sel[b * 8 + 2 * m + 1, c0 + 64:c0 + 128] = 1.0
    shared["selh"] = sel
    o32 = np.zeros((128, 512), np.float16)
    for b in range(B):
        for m in range(4):
            c0 = (b * 4 + m) * 32
            o32[0:64, c0 + b * 8 + 2 * m] = 1.0
            o32[64:128, c0 + b * 8 + 2 * m + 1] = 1.0
    shared["ones32"] = o32
    in_maps = []
    for k in range(N_CORES):
        i0 = k * IPC
        m = dict(shared)
        m["x"] = np.ascontiguousarray(
            x[:, i0:i0 + IPC].reshape(B * IPC * SD, T)).astype(np.float16)
        m["fw"] = np.ascontiguousarray(
            fw[i0:i0 + IPC].reshape(IPC * SD, T)).astype(np.float16)
        # booster output is sharded over j (rev[b,i,j]=bw[i,j]*hr[b,j]):
        # rows (j_loc, i) so the broadcast h row per 64-row group is local
        m["bw"] = np.ascontiguousarray(
            bw[:, i0:i0 + IPC].transpose(1, 0, 2).reshape(
                IPC * SD, T)).astype(np.float16)
        in_maps.append(m)
    return in_maps


def _get_nc():
    if "nc" not in _CACHE:
        _CACHE["nc"] = _build_nc()
    return _CACHE["nc"]


def run(inputs, trace=False, **kw):
    nc = _get_nc()
    in_maps = _prep_host(inputs)
    res = run_bass_kernel_spmd(nc, in_maps, core_ids=list(range(N_CORES)),
                               trace=trace, **kw)
    full = np.empty((B, SD, SD, T), dtype=np.float32)
    for k in range(N_CORES):
        i0 = k * IPC
        co = res.results[k]["out"].astype(np.float32).reshape(B, IPC, SD, T)
        full[:, :, i0:i0 + IPC] = co.transpose(0, 2, 1, 3)
    return full.reshape(B, T, NE), res


def kernel(**inputs):
    out, _ = run(inputs)
    return out
